# revision 2
# baseline (speedup 1.0000x reference)
"""GCN SpMM kernel for Trainium2 (8 NeuronCores, SPMD) — batched-gather version.

Computation (see reference):
    h = x @ W.T
    deg[n] = #edges with col==n;  dis = rsqrt(deg) (0 where deg==0)
    out[r] = sum_{e: row_e==r} dis[row_e]*dis[col_e] * h[col_e]

Linearity: out = (segment_sum(dis_r*dis_c * x[c], r)) @ W.T.  dis[col] is
folded into x on the host (xs = x*dis, cast bf16); dis[row] is applied as a
per-partition scale on the output copy (the projection is emitted transposed
so dest rows are partitions).

Sharding: output rows (and their edges) split across 8 cores; xs replicated
to every core's DRAM at staging (the halo all-gather — sources are random).

Per-core device program:
  - edges sorted by (source-bank, dest); four 25000-row banks so bank-local
    source ids fit dma_gather's int16 indices
  - per bank, one InstDMAGatherAnt per GK chunks of 128 edges (amortizes the
    ~1us SWDGE fixed cost GK-fold vs one indirect DMA per chunk):
        g[p, c, :] = xs_bank[idx[c*128+p], :]
  - per chunk, a one-hot scatter matrix S[e, d] = (iota[d] == dst_e) is
    built either by one DVE op or (for load balance) by two Act-engine
    activations (|iota-dst| then relu(1-z)); chunks whose edges span a
    window boundary get a 256-wide S covering both windows; dst 384 masks
    padding slots
  - PE accumulates aggT[f, d] += g_c[e, f]^T @ S[:, half] in PSUM over a
    window's chunk slices; the projection matmul uses lhsT=agg, rhs=W^T so
    PSUM holds [dest, feat]; the Act-engine output copy applies the
    per-dest dis_row as its per-partition scale; one DMA writes the bf16
    result out (host upcasts).
"""

import sys

sys.path.insert(0, "/opt/trn_rl_repo")

import numpy as np
import ml_dtypes

import concourse.bass as bass
import concourse.mybir as mybir
import concourse.tile as tile
from concourse.alu_op_type import AluOpType
from concourse.bass_utils import run_bass_kernel_spmd
from concourse.vector_clock import ScopedClock

# ---------------------------------------------------------------------------
# Workaround: this walrus build rejects instructions with >1 sync wait
# ("Too many sync wait commands"). TileContext's kernel-tail drain can carry
# several; split them across multiple drain instructions.
_MAX_WAITS = 1


def _split_drain_and_barrier(self, tick_clock, wait_clock):
    nc = self.nc
    drain_inst = nc.sync.drain()
    wait_clock.add_sem_waits(
        drain_inst.ins, ScopedClock({None: tick_clock.global_clock})
    )
    si = drain_inst.ins.sync_info
    if si is not None and si.on_wait and len(si.on_wait) > _MAX_WAITS:
        waits = list(si.on_wait)
        si.on_wait = waits[:_MAX_WAITS]
        rest = waits[_MAX_WAITS:]
        while rest:
            d2 = nc.sync.drain()
            si2 = d2.ins.sync_info
            if si2 is None:
                d2.ins.sync_info = mybir.SyncInfo(
                    on_wait=rest[:_MAX_WAITS], on_update=[]
                )
            else:
                si2.on_wait = rest[:_MAX_WAITS]
            rest = rest[_MAX_WAITS:]
    nc.all_engine_barrier()
    assert self.sems is not None
    popped = nc._tile_sem_poison_stack.pop()
    assert popped is self._sem_poison
    nc.clear_and_free_semaphores(list(self.sems.allocated().values()))
    nc.all_engine_barrier()


tile.TileContext._drain_and_barrier = _split_drain_and_barrier

import concourse.bass_utils as _bass_utils

_orig_get_walrus_args = _bass_utils.get_walrus_args


def _patched_walrus_args(*args, **kwargs):
    return [
        "--dge-levels=io,spill_reload,scalar_dynamic_offset,vector_dynamic_offsets",
        *_orig_get_walrus_args(*args, **kwargs),
    ]


_bass_utils.get_walrus_args = _patched_walrus_args


def split_multi_waits(nc):
    """Split any instruction carrying >1 sem waits: move extra waits onto
    same-engine NOPs inserted immediately before it (engines execute their
    instructions in block order, so this is equivalent)."""
    n = 0
    for bb in nc.m.functions[0].blocks:
        new_insts = []
        for ins in bb.instructions:
            si = ins.sync_info
            if si is not None and si.on_wait and len(si.on_wait) > 1:
                waits = list(si.on_wait)
                for w in waits[:-1]:
                    n += 1
                    nop = mybir.InstNoOp(
                        name=f"waitsplit-{n}-{ins.name}",
                        sync_info=mybir.SyncInfo(on_wait=[w], on_update=[]),
                        bass_nofuse=True,
                        engine=ins.engine,
                    )
                    new_insts.append(nop)
                si.on_wait = waits[-1:]
            new_insts.append(ins)
        bb.instructions[:] = new_insts
    return n


# ---------------------------------------------------------------------------

N_NODES = 100000
F = 128
N_CORES = 8
ROWS_PER_CORE = N_NODES // N_CORES  # 12500
WIN = 128  # dest rows per window
P = 128  # edges per chunk
NBANK = 4
BANK = N_NODES // NBANK  # 25000 source rows per bank (< int16 max)
N_WIN = (ROWS_PER_CORE + WIN - 1) // WIN  # 98
GK = 8  # chunks per dma_gather instruction; 1024 descriptors is the ring max
SCRATCH = 16 * GK * 128  # SWDGE ring sized to one gather (16B/descriptor)
PAD_DST = 384.0  # miss value outside both iota ranges (exact in bf16)
ACT_STT_EVERY = 3  # every 3rd pure chunk's one-hot is built on the Act engine

BF16 = ml_dtypes.bfloat16


def host_prep(x, W, edge_index):
    """Shared chunk grid + per-core edge placement.

    Returns per-core input arrays and the shared static layout:
    chunks per bank, per-chunk window/width info, per-window matmul lists.
    """
    row = np.asarray(edge_index[0], dtype=np.int64)
    col = np.asarray(edge_index[1], dtype=np.int64)
    x = np.asarray(x, dtype=np.float32)
    W = np.asarray(W, dtype=np.float32)

    deg = np.bincount(col, minlength=N_NODES)
    with np.errstate(divide="ignore"):
        dis = np.where(deg > 0, 1.0 / np.sqrt(deg.astype(np.float64)), 0.0).astype(
            np.float32
        )
    xs = (x * dis[:, None]).astype(BF16)  # dis[col] folded into sources

    core_of = row // ROWS_PER_CORE
    cnt = np.zeros((N_CORES, NBANK, N_WIN), dtype=np.int64)
    per_core = []
    for k in range(N_CORES):
        m = core_of == k
        r = (row[m] - k * ROWS_PER_CORE).astype(np.int64)
        c = col[m].astype(np.int64)
        b = c // BANK
        w = r >> 7
        order = np.lexsort((w, b))
        r, c, b, w = r[order], c[order], b[order], w[order]
        cnt[k] = np.stack(
            [np.bincount(w[b == bb], minlength=N_WIN) for bb in range(NBANK)]
        )
        per_core.append((r, c, b, w))

    cap = cnt.max(axis=0)  # [NBANK, N_WIN] shared capacity per (bank, window)
    pos = np.zeros((NBANK, N_WIN + 1), dtype=np.int64)
    pos[:, 1:] = np.cumsum(cap, axis=1)
    bank_tot = pos[:, -1]
    chunks_b = ((bank_tot + P - 1) // P).astype(np.int64)
    slots_b = chunks_b * P

    # shared static chunk info: first window + wide flag, per (bank, chunk)
    first_win = []
    is_wide = []
    for bb in range(NBANK):
        fw = np.searchsorted(pos[bb], np.arange(chunks_b[bb]) * P, side="right") - 1
        fw = np.minimum(fw, N_WIN - 1)
        lw = (
            np.searchsorted(
                pos[bb], np.arange(chunks_b[bb]) * P + (P - 1), side="right"
            )
            - 1
        )
        lw = np.minimum(lw, N_WIN - 1)
        assert (lw - fw <= 1).all(), "chunk spans >2 windows"
        first_win.append(fw)
        is_wide.append(lw > fw)

    # per-window matmul segment lists: (bank, chunk, half)
    segments = [[] for _ in range(N_WIN)]
    for bb in range(NBANK):
        for cch in range(int(chunks_b[bb])):
            w0 = int(first_win[bb][cch])
            segments[w0].append((bb, cch, 0))
            if is_wide[bb][cch]:
                segments[w0 + 1].append((bb, cch, 1))

    # per-core edge placement into the shared grid
    c_total = int(chunks_b.sum())
    c_off = np.concatenate([[0], np.cumsum(chunks_b)[:-1]])  # bank -> col base
    idx16 = [np.zeros((N_CORES, int(s)), dtype=np.int16) for s in slots_b]
    dstcol = np.full((N_CORES, P, c_total), PAD_DST, dtype=np.float32)
    for k in range(N_CORES):
        r, c, b, w = per_core[k]
        grp = b * N_WIN + w
        starts = np.zeros(NBANK * N_WIN, dtype=np.int64)
        gcnt = np.bincount(grp, minlength=NBANK * N_WIN)
        starts[1:] = np.cumsum(gcnt)[:-1]
        rank = np.arange(len(r)) - starts[grp]
        p = pos[b, w] + rank  # bank-local slot
        chunk = p >> 7
        part = p & (P - 1)
        fw_e = np.concatenate(first_win)[c_off[b] + chunk]
        dv = (r & (WIN - 1)) + 128 * (w - fw_e)
        assert ((w - fw_e) >= 0).all() and ((w - fw_e) <= 1).all()
        for bb in range(NBANK):
            m = b == bb
            idx16[bb][k, p[m]] = (c[m] - bb * BANK).astype(np.int16)
            dstcol[k, part[m], c_off[bb] + chunk[m]] = dv[m]

    # dma_gather index layout: idx i of a gather at 128-aligned offset o
    # lives at [partition (i-o)%16, slot (i-o)//16]; one global 16-wrap
    # layout serves every GK-chunk slice; replicate across 128 partitions.
    idx_tiles = []
    for bb in range(NBANK):
        s = int(slots_b[bb])
        t = idx16[bb].reshape(N_CORES, s // 16, 16).transpose(0, 2, 1)
        idx_tiles.append(np.ascontiguousarray(np.tile(t, (1, 8, 1))))

    dstcol_bf = dstcol.astype(BF16)
    dstcol_neg = np.ascontiguousarray(-dstcol)  # f32 bias for the Act path

    # dis of each core's dest rows, [partition(=dest-in-window), window]
    disrT = np.zeros((N_CORES, P, N_WIN), dtype=np.float32)
    for k in range(N_CORES):
        d = dis[k * ROWS_PER_CORE : (k + 1) * ROWS_PER_CORE]
        dpad = np.zeros(N_WIN * WIN, dtype=np.float32)
        dpad[: ROWS_PER_CORE] = d
        disrT[k] = dpad.reshape(N_WIN, WIN).T

    wt = np.ascontiguousarray(W.T)  # rhs[f, of] = W[of, f] for the transposed proj
    iota = np.tile(np.arange(2 * WIN, dtype=np.float32), (P, 1)).astype(BF16)
    layout = dict(
        chunks_b=[int(v) for v in chunks_b],
        c_off=[int(v) for v in c_off],
        first_win=[fw.tolist() for fw in first_win],
        is_wide=[iw.tolist() for iw in is_wide],
        segments=segments,
        c_total=c_total,
    )
    return xs, wt, iota, idx_tiles, dstcol_bf, dstcol_neg, disrT, layout


def build_program(layout):
    """Build the per-core Bass/Tile program (same for all cores)."""
    nc = bass.Bass(
        "TRN2",
        target_bir_lowering=False,
        debug=False,
        num_devices=1,
        dynamic_dma_scratch_size=SCRATCH,
    )
    dt = mybir.dt
    AF = mybir.ActivationFunctionType
    chunks_b = layout["chunks_b"]
    c_off = layout["c_off"]
    is_wide = layout["is_wide"]
    segments = layout["segments"]
    c_total = layout["c_total"]

    xs_d = [
        nc.dram_tensor(f"xs{b}", [BANK, F], dt.bfloat16, kind="ExternalInput")
        for b in range(NBANK)
    ]
    wt_d = nc.dram_tensor("wt", [F, F], dt.float32, kind="ExternalInput")
    iota_d = nc.dram_tensor("iota", [P, 2 * WIN], dt.bfloat16, kind="ExternalInput")
    idx_d = [
        nc.dram_tensor(
            f"idx{b}", [P, chunks_b[b] * P // 16], dt.int16, kind="ExternalInput"
        )
        for b in range(NBANK)
    ]
    sdst_d = nc.dram_tensor("sdst", [P, c_total], dt.bfloat16, kind="ExternalInput")
    sdstn_d = nc.dram_tensor("sdstn", [P, c_total], dt.float32, kind="ExternalInput")
    disr_d = nc.dram_tensor("disr", [P, N_WIN], dt.float32, kind="ExternalInput")
    y_d = nc.dram_tensor("y", [P, N_WIN * F], dt.bfloat16, kind="ExternalOutput")

    n_batch = [(chunks_b[b] + GK - 1) // GK for b in range(NBANK)]

    with tile.TileContext(nc) as tc:
        with (
            tc.tile_pool(name="const", bufs=1) as const_pool,
            tc.tile_pool(name="out", bufs=1) as out_pool,
            tc.tile_pool(name="g0", bufs=4) as g_pool0,
            tc.tile_pool(name="g1", bufs=4) as g_pool1,
            tc.tile_pool(name="g2", bufs=4) as g_pool2,
            tc.tile_pool(name="g3", bufs=4) as g_pool3,
            tc.tile_pool(name="sp", bufs=10) as sp_pool,
            tc.tile_pool(name="sw", bufs=6) as sw_pool,
            tc.tile_pool(name="pt", bufs=4) as pt_pool,
            tc.tile_pool(name="aggsb", bufs=3) as aggsb_pool,
            tc.tile_pool(name="psum_agg", bufs=4, space="PSUM") as pa_pool,
            tc.tile_pool(name="psum_proj", bufs=2, space="PSUM") as pp_pool,
        ):
            from concourse.library_config import mlp

            nc.gpsimd.load_library(mlp)
            g_pools = [g_pool0, g_pool1, g_pool2, g_pool3]
            wt_sb = const_pool.tile([F, F], dt.float32, tag="wt")
            nc.sync.dma_start(out=wt_sb[:], in_=wt_d.ap())
            iota_sb = const_pool.tile([P, 2 * WIN], dt.bfloat16, tag="iota")
            nc.sync.dma_start(out=iota_sb[:], in_=iota_d.ap())
            sdst_sb = const_pool.tile([P, c_total], dt.bfloat16, tag="sdst")
            nc.sync.dma_start(out=sdst_sb[:], in_=sdst_d.ap())
            sdstn_sb = const_pool.tile([P, c_total], dt.float32, tag="sdstn")
            nc.sync.dma_start(out=sdstn_sb[:], in_=sdstn_d.ap())
            disr_sb = const_pool.tile([P, N_WIN], dt.float32, tag="disr")
            nc.sync.dma_start(out=disr_sb[:], in_=disr_d.ap())
            idx_sb = []
            for b in range(NBANK):
                t = const_pool.tile(
                    [P, chunks_b[b] * P // 16], dt.int16, tag=f"idx{b}"
                )
                nc.sync.dma_start(out=t[:], in_=idx_d[b].ap())
                idx_sb.append(t)
            out_sb = out_pool.tile([P, N_WIN * F], dt.bfloat16, tag="out")

            g_tiles = [[None] * n_batch[b] for b in range(NBANK)]
            s_tiles = {}  # (bank, chunk) -> tile
            reg_cache = {}
            pure_ctr = 0

            def nreg(v):
                if v not in reg_cache:
                    reg_cache[v] = nc.gpsimd.to_reg(v)
                return reg_cache[v]

            def emit_gather(b, bi):
                c_lo = bi * GK
                c_n = min(GK, chunks_b[b] - c_lo)
                g_t = g_pools[b].tile([P, GK, F], dt.bfloat16, tag=f"g{b}")
                nc.gpsimd.dma_gather(
                    g_t[:, :c_n, :],
                    xs_d[b].ap(),
                    idx_sb[b][:, c_lo * 8 : (c_lo + c_n) * 8],
                    c_n * P,
                    nreg(c_n * P),
                    F,
                )
                g_tiles[b][bi] = g_t

            def emit_s(b, cch):
                nonlocal pure_ctr
                wide = is_wide[b][cch]
                width = 2 * WIN if wide else WIN
                col = c_off[b] + cch
                on_act = False
                if not wide and ACT_STT_EVERY:
                    pure_ctr += 1
                    on_act = pure_ctr % ACT_STT_EVERY == 0
                pool_ = sw_pool if wide else sp_pool
                s_t = pool_.tile([P, width], dt.bfloat16, tag="sw" if wide else "sp")
                if on_act:
                    # |iota - dst| then relu(1 - z): exact one-hot for ints
                    t_t = pt_pool.tile([P, WIN], dt.bfloat16, tag="pt")
                    nc.scalar.activation(
                        out=t_t[:],
                        in_=iota_sb[:, :WIN],
                        func=AF.Abs,
                        bias=sdstn_sb[:, col : col + 1],
                    )
                    nc.scalar.activation(
                        out=s_t[:], in_=t_t[:], func=AF.Relu, bias=1.0, scale=-1.0
                    )
                else:
                    nc.vector.scalar_tensor_tensor(
                        out=s_t[:],
                        in0=iota_sb[:, :width],
                        scalar=sdst_sb[:, col : col + 1],
                        in1=iota_sb[:, :width],
                        op0=AluOpType.is_equal,
                        op1=AluOpType.bypass,
                    )
                s_tiles[(b, cch)] = s_t

            for w in range(N_WIN):
                segs = segments[w]
                if not segs:
                    continue
                agg = pa_pool.tile([F, WIN], dt.float32, tag="agg")
                n_s = len(segs)
                for j, (b, cch, half) in enumerate(segs):
                    bi, cb = divmod(cch, GK)
                    if g_tiles[b][bi] is None:
                        emit_gather(b, bi)
                    if half == 0:
                        emit_s(b, cch)
                    s_t = s_tiles[(b, cch)]
                    nc.tensor.matmul(
                        agg[:],
                        lhsT=g_tiles[b][bi][:, cb, :],
                        rhs=s_t[:, half * WIN : (half + 1) * WIN],
                        start=(j == 0),
                        stop=(j == n_s - 1),
                    )
                agg_sb = aggsb_pool.tile([F, WIN], dt.float32, tag="aggsb")
                nc.scalar.activation(out=agg_sb[:], in_=agg[:], func=AF.Copy)
                proj = pp_pool.tile([WIN, F], dt.float32, tag="proj")
                nc.tensor.matmul(
                    proj[:], lhsT=agg_sb[:], rhs=wt_sb[:], start=True, stop=True
                )
                # PSUM [dest, feat] -> out, scaled by dis_row (per-partition)
                nc.scalar.activation(
                    out=out_sb[:, w * F : (w + 1) * F],
                    in_=proj[:],
                    func=AF.Copy,
                    scale=disr_sb[:, w : w + 1],
                )
            nc.sync.dma_start(out=y_d.ap(), in_=out_sb[:])
    split_multi_waits(nc)
    mybir.codegen_inst_isa_subclasses(nc)
    return nc


def make_in_maps(xs, wt, iota, idx_tiles, dstcol, dstneg, disrT):
    in_maps = []
    for k in range(N_CORES):
        m = {
            "wt": wt,
            "iota": iota,
            "sdst": dstcol[k],
            "sdstn": dstneg[k],
            "disr": disrT[k],
        }
        for b in range(NBANK):
            m[f"xs{b}"] = xs[b * BANK : (b + 1) * BANK]
            m[f"idx{b}"] = idx_tiles[b][k]
        in_maps.append(m)
    return in_maps


def unshard(res):
    outs = []
    for k in range(N_CORES):
        y_t = res.results[k]["y"].astype(np.float32)  # [P, N_WIN*F]
        o = y_t.reshape(P, N_WIN, F).transpose(1, 0, 2).reshape(N_WIN * WIN, F)
        outs.append(o[:ROWS_PER_CORE])
    return np.concatenate(outs, axis=0)


def kernel(x, W, edge_index):
    xs, wt, iota, idx_tiles, dstcol, dstneg, disrT, layout = host_prep(
        x, W, edge_index
    )
    nc = build_program(layout)
    in_maps = make_in_maps(xs, wt, iota, idx_tiles, dstcol, dstneg, disrT)
    res = run_bass_kernel_spmd(nc, in_maps, core_ids=list(range(N_CORES)))
    return unshard(res)


# revision 4
# speedup vs baseline: 1.0530x; 1.0530x over previous
"""GCN SpMM kernel for Trainium2 (8 NeuronCores, SPMD) — batched-gather version.

Computation (see reference):
    h = x @ W.T
    deg[n] = #edges with col==n;  dis = rsqrt(deg) (0 where deg==0)
    out[r] = sum_{e: row_e==r} dis[row_e]*dis[col_e] * h[col_e]

Linearity: out = (segment_sum(dis_r*dis_c * x[c], r)) @ W.T.  dis[col] is
folded into x on the host (xs = x*dis, cast bf16); dis[row] is applied as a
per-partition scale on the output copy (the projection is emitted transposed
so dest rows are partitions).

Sharding: output rows (and their edges) split across 8 cores; xs replicated
to every core's DRAM at staging (the halo all-gather — sources are random).

Per-core device program:
  - edges sorted by (source-bank, dest); four 25000-row banks so bank-local
    source ids fit dma_gather's int16 indices
  - per bank, one InstDMAGatherAnt per GK chunks of 128 edges (amortizes the
    ~1us SWDGE fixed cost GK-fold vs one indirect DMA per chunk):
        g[p, c, :] = xs_bank[idx[c*128+p], :]
  - per chunk, a one-hot scatter matrix S[e, d] = (iota[d] == dst_e) is
    built either by one DVE op or (for load balance) by two Act-engine
    activations (|iota-dst| then relu(1-z)); chunks whose edges span a
    window boundary get a 256-wide S covering both windows; dst 384 masks
    padding slots
  - PE accumulates aggT[f, d] += g_c[e, f]^T @ S[:, half] in PSUM over a
    window's chunk slices; the projection matmul uses lhsT=agg, rhs=W^T so
    PSUM holds [dest, feat]; the Act-engine output copy applies the
    per-dest dis_row as its per-partition scale; one DMA writes the bf16
    result out (host upcasts).
"""

import sys

sys.path.insert(0, "/opt/trn_rl_repo")

import numpy as np
import ml_dtypes

import concourse.bass as bass
import concourse.mybir as mybir
import concourse.tile as tile
from concourse.alu_op_type import AluOpType
from concourse.bass_utils import run_bass_kernel_spmd
from concourse.vector_clock import ScopedClock

# ---------------------------------------------------------------------------
# Workaround: this walrus build rejects instructions with >1 sync wait
# ("Too many sync wait commands"). TileContext's kernel-tail drain can carry
# several; split them across multiple drain instructions.
_MAX_WAITS = 1


def _split_drain_and_barrier(self, tick_clock, wait_clock):
    nc = self.nc
    drain_inst = nc.sync.drain()
    wait_clock.add_sem_waits(
        drain_inst.ins, ScopedClock({None: tick_clock.global_clock})
    )
    si = drain_inst.ins.sync_info
    if si is not None and si.on_wait and len(si.on_wait) > _MAX_WAITS:
        waits = list(si.on_wait)
        si.on_wait = waits[:_MAX_WAITS]
        rest = waits[_MAX_WAITS:]
        while rest:
            d2 = nc.sync.drain()
            si2 = d2.ins.sync_info
            if si2 is None:
                d2.ins.sync_info = mybir.SyncInfo(
                    on_wait=rest[:_MAX_WAITS], on_update=[]
                )
            else:
                si2.on_wait = rest[:_MAX_WAITS]
            rest = rest[_MAX_WAITS:]
    nc.all_engine_barrier()
    assert self.sems is not None
    popped = nc._tile_sem_poison_stack.pop()
    assert popped is self._sem_poison
    nc.clear_and_free_semaphores(list(self.sems.allocated().values()))
    nc.all_engine_barrier()


tile.TileContext._drain_and_barrier = _split_drain_and_barrier

import concourse.bass_utils as _bass_utils

_orig_get_walrus_args = _bass_utils.get_walrus_args


def _patched_walrus_args(*args, **kwargs):
    return [
        "--dge-levels=io,spill_reload,scalar_dynamic_offset,vector_dynamic_offsets",
        *_orig_get_walrus_args(*args, **kwargs),
    ]


_bass_utils.get_walrus_args = _patched_walrus_args


def split_multi_waits(nc):
    """Split any instruction carrying >1 sem waits: move extra waits onto
    same-engine NOPs inserted immediately before it (engines execute their
    instructions in block order, so this is equivalent)."""
    n = 0
    for bb in nc.m.functions[0].blocks:
        new_insts = []
        for ins in bb.instructions:
            si = ins.sync_info
            if si is not None and si.on_wait and len(si.on_wait) > 1:
                waits = list(si.on_wait)
                for w in waits[:-1]:
                    n += 1
                    nop = mybir.InstNoOp(
                        name=f"waitsplit-{n}-{ins.name}",
                        sync_info=mybir.SyncInfo(on_wait=[w], on_update=[]),
                        bass_nofuse=True,
                        engine=ins.engine,
                    )
                    new_insts.append(nop)
                si.on_wait = waits[-1:]
            new_insts.append(ins)
        bb.instructions[:] = new_insts
    return n


# ---------------------------------------------------------------------------

N_NODES = 100000
F = 128
N_CORES = 8
ROWS_PER_CORE = N_NODES // N_CORES  # 12500
WIN = 128  # dest rows per window
P = 128  # edges per chunk
NBANK = 4
BANK = N_NODES // NBANK  # 25000 source rows per bank (< int16 max)
N_WIN = (ROWS_PER_CORE + WIN - 1) // WIN  # 98
GK = 8  # chunks per dma_gather instruction; 1024 descriptors is the ring max
SCRATCH = max(16384, 16 * GK * 128)  # SWDGE ring (16B/descriptor)
PAD_DST = 384.0  # miss value outside both iota ranges (exact in bf16)
ACT_STT_EVERY = 3  # every 3rd pure chunk's one-hot is built on the Act engine
G_BUFS = 4
SP_BUFS = 10
SW_BUFS = 6
PT_BUFS = 4
AGG_BUFS = 3
PA_BUFS = 4
PP_BUFS = 2

BF16 = ml_dtypes.bfloat16


def host_prep(x, W, edge_index):
    """Shared chunk grid + per-core edge placement.

    Returns per-core input arrays and the shared static layout:
    chunks per bank, per-chunk window/width info, per-window matmul lists.
    """
    row = np.asarray(edge_index[0], dtype=np.int64)
    col = np.asarray(edge_index[1], dtype=np.int64)
    x = np.asarray(x, dtype=np.float32)
    W = np.asarray(W, dtype=np.float32)

    deg = np.bincount(col, minlength=N_NODES)
    with np.errstate(divide="ignore"):
        dis = np.where(deg > 0, 1.0 / np.sqrt(deg.astype(np.float64)), 0.0).astype(
            np.float32
        )
    xs = (x * dis[:, None]).astype(BF16)  # dis[col] folded into sources

    core_of = row // ROWS_PER_CORE
    cnt = np.zeros((N_CORES, NBANK, N_WIN), dtype=np.int64)
    per_core = []
    rowmap = np.full((N_CORES, N_WIN * WIN), -1, dtype=np.int64)
    for k in range(N_CORES):
        m = core_of == k
        r = (row[m] - k * ROWS_PER_CORE).astype(np.int64)
        c = col[m].astype(np.int64)
        b = c // BANK
        # balance rows across windows so per-(bank, window) edge counts are
        # nearly equal on every core (shrinks the cross-core max padding):
        # greedy argmin of correlation with current window-bank sums.
        ebc = np.zeros((ROWS_PER_CORE, NBANK), dtype=np.int64)
        np.add.at(ebc, (r, b), 1)
        tot = ebc.sum(axis=1)
        order_rows = np.argsort(-tot, kind="stable")
        sums = np.zeros((N_WIN, NBANK), dtype=np.float64)
        space = np.full(N_WIN, WIN, dtype=np.int64)
        win_of = np.zeros(ROWS_PER_CORE, dtype=np.int64)
        slot_of = np.zeros(ROWS_PER_CORE, dtype=np.int64)
        for rr in order_rows:
            e = ebc[rr].astype(np.float64)
            score = sums @ e + 0.5 * (e @ e)
            score[space == 0] = np.inf
            wbest = int(np.argmin(score))
            win_of[rr] = wbest
            slot_of[rr] = WIN - space[wbest]
            sums[wbest] += e
            space[wbest] -= 1
            rowmap[k, wbest * WIN + slot_of[rr]] = k * ROWS_PER_CORE + rr
        w = win_of[r]
        dst_in_w = slot_of[r]
        order = np.lexsort((w, b))
        r, c, b, w, dst_in_w = (
            r[order], c[order], b[order], w[order], dst_in_w[order]
        )
        cnt[k] = np.stack(
            [np.bincount(w[b == bb], minlength=N_WIN) for bb in range(NBANK)]
        )
        per_core.append((r, c, b, w, dst_in_w))

    cap = cnt.max(axis=0)  # [NBANK, N_WIN] shared capacity per (bank, window)
    pos = np.zeros((NBANK, N_WIN + 1), dtype=np.int64)
    pos[:, 1:] = np.cumsum(cap, axis=1)
    bank_tot = pos[:, -1]
    chunks_b = ((bank_tot + P - 1) // P).astype(np.int64)
    slots_b = chunks_b * P

    # shared static chunk info: first window + wide flag, per (bank, chunk)
    first_win = []
    is_wide = []
    for bb in range(NBANK):
        fw = np.searchsorted(pos[bb], np.arange(chunks_b[bb]) * P, side="right") - 1
        fw = np.minimum(fw, N_WIN - 1)
        lw = (
            np.searchsorted(
                pos[bb], np.arange(chunks_b[bb]) * P + (P - 1), side="right"
            )
            - 1
        )
        lw = np.minimum(lw, N_WIN - 1)
        assert (lw - fw <= 1).all(), "chunk spans >2 windows"
        first_win.append(fw)
        is_wide.append(lw > fw)

    # per-window matmul segment lists: (bank, chunk, half)
    segments = [[] for _ in range(N_WIN)]
    for bb in range(NBANK):
        for cch in range(int(chunks_b[bb])):
            w0 = int(first_win[bb][cch])
            segments[w0].append((bb, cch, 0))
            if is_wide[bb][cch]:
                segments[w0 + 1].append((bb, cch, 1))

    # per-core edge placement into the shared grid
    c_total = int(chunks_b.sum())
    c_off = np.concatenate([[0], np.cumsum(chunks_b)[:-1]])  # bank -> col base
    idx16 = [np.zeros((N_CORES, int(s)), dtype=np.int16) for s in slots_b]
    dstcol = np.full((N_CORES, P, c_total), PAD_DST, dtype=np.float32)
    for k in range(N_CORES):
        r, c, b, w, dst_in_w = per_core[k]
        grp = b * N_WIN + w
        starts = np.zeros(NBANK * N_WIN, dtype=np.int64)
        gcnt = np.bincount(grp, minlength=NBANK * N_WIN)
        starts[1:] = np.cumsum(gcnt)[:-1]
        rank = np.arange(len(r)) - starts[grp]
        p = pos[b, w] + rank  # bank-local slot
        chunk = p >> 7
        part = p & (P - 1)
        fw_e = np.concatenate(first_win)[c_off[b] + chunk]
        dv = dst_in_w + 128 * (w - fw_e)
        assert ((w - fw_e) >= 0).all() and ((w - fw_e) <= 1).all()
        for bb in range(NBANK):
            m = b == bb
            idx16[bb][k, p[m]] = (c[m] - bb * BANK).astype(np.int16)
            dstcol[k, part[m], c_off[bb] + chunk[m]] = dv[m]

    # dma_gather index layout: idx i of a gather at 128-aligned offset o
    # lives at [partition (i-o)%16, slot (i-o)//16]; one global 16-wrap
    # layout serves every GK-chunk slice; replicate across 128 partitions.
    idx_tiles = []
    for bb in range(NBANK):
        s = int(slots_b[bb])
        t = idx16[bb].reshape(N_CORES, s // 16, 16).transpose(0, 2, 1)
        idx_tiles.append(np.ascontiguousarray(np.tile(t, (1, 8, 1))))

    dstcol_bf = dstcol.astype(BF16)
    dstcol_neg = np.ascontiguousarray(-dstcol)  # f32 bias for the Act path

    # dis of each core's dest rows, [partition(=slot-in-window), window]
    disrT = np.zeros((N_CORES, P, N_WIN), dtype=np.float32)
    for k in range(N_CORES):
        dpad = np.zeros(N_WIN * WIN, dtype=np.float32)
        mm = rowmap[k] >= 0
        dpad[mm] = dis[rowmap[k][mm]]
        disrT[k] = dpad.reshape(N_WIN, WIN).T

    wt = np.ascontiguousarray(W.T)  # rhs[f, of] = W[of, f] for the transposed proj
    iota = np.tile(np.arange(2 * WIN, dtype=np.float32), (P, 1)).astype(BF16)
    layout = dict(
        chunks_b=[int(v) for v in chunks_b],
        c_off=[int(v) for v in c_off],
        first_win=[fw.tolist() for fw in first_win],
        is_wide=[iw.tolist() for iw in is_wide],
        segments=segments,
        c_total=c_total,
    )
    return xs, wt, iota, idx_tiles, dstcol_bf, dstcol_neg, disrT, rowmap, layout


def build_program(layout):
    """Build the per-core Bass/Tile program (same for all cores)."""
    nc = bass.Bass(
        "TRN2",
        target_bir_lowering=False,
        debug=False,
        num_devices=1,
        dynamic_dma_scratch_size=SCRATCH,
    )
    dt = mybir.dt
    AF = mybir.ActivationFunctionType
    chunks_b = layout["chunks_b"]
    c_off = layout["c_off"]
    is_wide = layout["is_wide"]
    segments = layout["segments"]
    c_total = layout["c_total"]

    xs_d = [
        nc.dram_tensor(f"xs{b}", [BANK, F], dt.bfloat16, kind="ExternalInput")
        for b in range(NBANK)
    ]
    wt_d = nc.dram_tensor("wt", [F, F], dt.float32, kind="ExternalInput")
    iota_d = nc.dram_tensor("iota", [P, 2 * WIN], dt.bfloat16, kind="ExternalInput")
    idx_d = [
        nc.dram_tensor(
            f"idx{b}", [P, chunks_b[b] * P // 16], dt.int16, kind="ExternalInput"
        )
        for b in range(NBANK)
    ]
    sdst_d = nc.dram_tensor("sdst", [P, c_total], dt.bfloat16, kind="ExternalInput")
    sdstn_d = nc.dram_tensor("sdstn", [P, c_total], dt.float32, kind="ExternalInput")
    disr_d = nc.dram_tensor("disr", [P, N_WIN], dt.float32, kind="ExternalInput")
    y_d = nc.dram_tensor("y", [P, N_WIN * F], dt.bfloat16, kind="ExternalOutput")

    n_batch = [(chunks_b[b] + GK - 1) // GK for b in range(NBANK)]

    with tile.TileContext(nc) as tc:
        with (
            tc.tile_pool(name="const", bufs=1) as const_pool,
            tc.tile_pool(name="out", bufs=1) as out_pool,
            tc.tile_pool(name="g0", bufs=G_BUFS) as g_pool0,
            tc.tile_pool(name="g1", bufs=G_BUFS) as g_pool1,
            tc.tile_pool(name="g2", bufs=G_BUFS) as g_pool2,
            tc.tile_pool(name="g3", bufs=G_BUFS) as g_pool3,
            tc.tile_pool(name="sp", bufs=SP_BUFS) as sp_pool,
            tc.tile_pool(name="sw", bufs=SW_BUFS) as sw_pool,
            tc.tile_pool(name="pt", bufs=PT_BUFS) as pt_pool,
            tc.tile_pool(name="aggsb", bufs=AGG_BUFS) as aggsb_pool,
            tc.tile_pool(name="psum_agg", bufs=PA_BUFS, space="PSUM") as pa_pool,
            tc.tile_pool(name="psum_proj", bufs=PP_BUFS, space="PSUM") as pp_pool,
        ):
            from concourse.library_config import mlp

            nc.gpsimd.load_library(mlp)
            g_pools = [g_pool0, g_pool1, g_pool2, g_pool3]
            wt_sb = const_pool.tile([F, F], dt.float32, tag="wt")
            nc.sync.dma_start(out=wt_sb[:], in_=wt_d.ap())
            iota_sb = const_pool.tile([P, 2 * WIN], dt.bfloat16, tag="iota")
            nc.sync.dma_start(out=iota_sb[:], in_=iota_d.ap())
            sdst_sb = const_pool.tile([P, c_total], dt.bfloat16, tag="sdst")
            nc.sync.dma_start(out=sdst_sb[:], in_=sdst_d.ap())
            sdstn_sb = const_pool.tile([P, c_total], dt.float32, tag="sdstn")
            nc.sync.dma_start(out=sdstn_sb[:], in_=sdstn_d.ap())
            disr_sb = const_pool.tile([P, N_WIN], dt.float32, tag="disr")
            nc.sync.dma_start(out=disr_sb[:], in_=disr_d.ap())
            idx_sb = []
            for b in range(NBANK):
                t = const_pool.tile(
                    [P, chunks_b[b] * P // 16], dt.int16, tag=f"idx{b}"
                )
                nc.sync.dma_start(out=t[:], in_=idx_d[b].ap())
                idx_sb.append(t)
            out_sb = out_pool.tile([P, N_WIN * F], dt.bfloat16, tag="out")

            g_tiles = [[None] * n_batch[b] for b in range(NBANK)]
            s_tiles = {}  # (bank, chunk) -> tile
            reg_cache = {}
            pure_ctr = 0

            def nreg(v):
                if v not in reg_cache:
                    reg_cache[v] = nc.gpsimd.to_reg(v)
                return reg_cache[v]

            def emit_gather(b, bi):
                c_lo = bi * GK
                c_n = min(GK, chunks_b[b] - c_lo)
                g_t = g_pools[b].tile([P, GK, F], dt.bfloat16, tag=f"g{b}")
                nc.gpsimd.dma_gather(
                    g_t[:, :c_n, :],
                    xs_d[b].ap(),
                    idx_sb[b][:, c_lo * 8 : (c_lo + c_n) * 8],
                    c_n * P,
                    nreg(c_n * P),
                    F,
                )
                g_tiles[b][bi] = g_t

            def emit_s(b, cch):
                nonlocal pure_ctr
                wide = is_wide[b][cch]
                width = 2 * WIN if wide else WIN
                col = c_off[b] + cch
                on_act = False
                if not wide and ACT_STT_EVERY:
                    pure_ctr += 1
                    on_act = pure_ctr % ACT_STT_EVERY == 0
                pool_ = sw_pool if wide else sp_pool
                s_t = pool_.tile([P, width], dt.bfloat16, tag="sw" if wide else "sp")
                if on_act:
                    # |iota - dst| then relu(1 - z): exact one-hot for ints
                    t_t = pt_pool.tile([P, WIN], dt.bfloat16, tag="pt")
                    nc.scalar.activation(
                        out=t_t[:],
                        in_=iota_sb[:, :WIN],
                        func=AF.Abs,
                        bias=sdstn_sb[:, col : col + 1],
                    )
                    nc.scalar.activation(
                        out=s_t[:], in_=t_t[:], func=AF.Relu, bias=1.0, scale=-1.0
                    )
                else:
                    nc.vector.scalar_tensor_tensor(
                        out=s_t[:],
                        in0=iota_sb[:, :width],
                        scalar=sdst_sb[:, col : col + 1],
                        in1=iota_sb[:, :width],
                        op0=AluOpType.is_equal,
                        op1=AluOpType.bypass,
                    )
                s_tiles[(b, cch)] = s_t

            for w in range(N_WIN):
                segs = segments[w]
                if not segs:
                    continue
                agg = pa_pool.tile([F, WIN], dt.float32, tag="agg")
                n_s = len(segs)
                for j, (b, cch, half) in enumerate(segs):
                    bi, cb = divmod(cch, GK)
                    if g_tiles[b][bi] is None:
                        emit_gather(b, bi)
                    if half == 0:
                        emit_s(b, cch)
                    s_t = s_tiles[(b, cch)]
                    nc.tensor.matmul(
                        agg[:],
                        lhsT=g_tiles[b][bi][:, cb, :],
                        rhs=s_t[:, half * WIN : (half + 1) * WIN],
                        start=(j == 0),
                        stop=(j == n_s - 1),
                    )
                agg_sb = aggsb_pool.tile([F, WIN], dt.float32, tag="aggsb")
                nc.scalar.activation(out=agg_sb[:], in_=agg[:], func=AF.Copy)
                proj = pp_pool.tile([WIN, F], dt.float32, tag="proj")
                nc.tensor.matmul(
                    proj[:], lhsT=agg_sb[:], rhs=wt_sb[:], start=True, stop=True
                )
                # PSUM [dest, feat] -> out, scaled by dis_row (per-partition)
                nc.scalar.activation(
                    out=out_sb[:, w * F : (w + 1) * F],
                    in_=proj[:],
                    func=AF.Copy,
                    scale=disr_sb[:, w : w + 1],
                )
            nc.sync.dma_start(out=y_d.ap(), in_=out_sb[:])
    split_multi_waits(nc)
    mybir.codegen_inst_isa_subclasses(nc)
    return nc


def make_in_maps(xs, wt, iota, idx_tiles, dstcol, dstneg, disrT):
    in_maps = []
    for k in range(N_CORES):
        m = {
            "wt": wt,
            "iota": iota,
            "sdst": dstcol[k],
            "sdstn": dstneg[k],
            "disr": disrT[k],
        }
        for b in range(NBANK):
            m[f"xs{b}"] = xs[b * BANK : (b + 1) * BANK]
            m[f"idx{b}"] = idx_tiles[b][k]
        in_maps.append(m)
    return in_maps


def unshard(res, rowmap):
    out = np.zeros((N_NODES, F), dtype=np.float32)
    for k in range(N_CORES):
        y_t = res.results[k]["y"].astype(np.float32)  # [P, N_WIN*F]
        o = y_t.reshape(P, N_WIN, F).transpose(1, 0, 2).reshape(N_WIN * WIN, F)
        mm = rowmap[k] >= 0
        out[rowmap[k][mm]] = o[mm]
    return out


def kernel(x, W, edge_index):
    xs, wt, iota, idx_tiles, dstcol, dstneg, disrT, rowmap, layout = host_prep(
        x, W, edge_index
    )
    nc = build_program(layout)
    in_maps = make_in_maps(xs, wt, iota, idx_tiles, dstcol, dstneg, disrT)
    res = run_bass_kernel_spmd(nc, in_maps, core_ids=list(range(N_CORES)))
    return unshard(res, rowmap)


# revision 7
# speedup vs baseline: 1.0854x; 1.0308x over previous
"""GCN SpMM kernel for Trainium2 (8 NeuronCores, SPMD) — batched-gather version.

Computation (see reference):
    h = x @ W.T
    deg[n] = #edges with col==n;  dis = rsqrt(deg) (0 where deg==0)
    out[r] = sum_{e: row_e==r} dis[row_e]*dis[col_e] * h[col_e]

Linearity: out = (segment_sum(dis_r*dis_c * x[c], r)) @ W.T.  dis[col] is
folded into x on the host (xs = x*dis, cast bf16); dis[row] is applied as a
per-partition scale on the output copy (the projection is emitted transposed
so dest rows are partitions).

Sharding: output rows (and their edges) split across 8 cores; xs replicated
to every core's DRAM at staging (the halo all-gather — sources are random).

Per-core device program:
  - edges sorted by (source-bank, dest); four 25000-row banks so bank-local
    source ids fit dma_gather's int16 indices
  - per bank, one InstDMAGatherAnt per GK chunks of 128 edges (amortizes the
    ~1us SWDGE fixed cost GK-fold vs one indirect DMA per chunk):
        g[p, c, :] = xs_bank[idx[c*128+p], :]
  - per chunk, a one-hot scatter matrix S[e, d] = (iota[d] == dst_e) is
    built either by one DVE op or (for load balance) by two Act-engine
    activations (|iota-dst| then relu(1-z)); chunks whose edges span a
    window boundary get a 256-wide S covering both windows; dst 384 masks
    padding slots
  - PE accumulates aggT[f, d] += g_c[e, f]^T @ S[:, half] in PSUM over a
    window's chunk slices; the projection matmul uses lhsT=agg, rhs=W^T so
    PSUM holds [dest, feat]; the Act-engine output copy applies the
    per-dest dis_row as its per-partition scale; one DMA writes the bf16
    result out (host upcasts).
"""

import sys

sys.path.insert(0, "/opt/trn_rl_repo")

import numpy as np
import ml_dtypes

import concourse.bass as bass
import concourse.mybir as mybir
import concourse.tile as tile
from concourse.alu_op_type import AluOpType
from concourse.bass_utils import run_bass_kernel_spmd
from concourse.vector_clock import ScopedClock

# ---------------------------------------------------------------------------
# Workaround: this walrus build rejects instructions with >1 sync wait
# ("Too many sync wait commands"). TileContext's kernel-tail drain can carry
# several; split them across multiple drain instructions.
_MAX_WAITS = 1


def _split_drain_and_barrier(self, tick_clock, wait_clock):
    nc = self.nc
    drain_inst = nc.sync.drain()
    wait_clock.add_sem_waits(
        drain_inst.ins, ScopedClock({None: tick_clock.global_clock})
    )
    si = drain_inst.ins.sync_info
    if si is not None and si.on_wait and len(si.on_wait) > _MAX_WAITS:
        waits = list(si.on_wait)
        si.on_wait = waits[:_MAX_WAITS]
        rest = waits[_MAX_WAITS:]
        while rest:
            d2 = nc.sync.drain()
            si2 = d2.ins.sync_info
            if si2 is None:
                d2.ins.sync_info = mybir.SyncInfo(
                    on_wait=rest[:_MAX_WAITS], on_update=[]
                )
            else:
                si2.on_wait = rest[:_MAX_WAITS]
            rest = rest[_MAX_WAITS:]
    nc.all_engine_barrier()
    assert self.sems is not None
    popped = nc._tile_sem_poison_stack.pop()
    assert popped is self._sem_poison
    nc.clear_and_free_semaphores(list(self.sems.allocated().values()))
    nc.all_engine_barrier()


tile.TileContext._drain_and_barrier = _split_drain_and_barrier

import concourse.bass_utils as _bass_utils

_orig_get_walrus_args = _bass_utils.get_walrus_args


def _patched_walrus_args(*args, **kwargs):
    return [
        "--dge-levels=io,spill_reload,scalar_dynamic_offset,vector_dynamic_offsets",
        *_orig_get_walrus_args(*args, **kwargs),
    ]


_bass_utils.get_walrus_args = _patched_walrus_args


def split_multi_waits(nc):
    """Split any instruction carrying >1 sem waits: move extra waits onto
    same-engine NOPs inserted immediately before it (engines execute their
    instructions in block order, so this is equivalent)."""
    n = 0
    for bb in nc.m.functions[0].blocks:
        new_insts = []
        for ins in bb.instructions:
            si = ins.sync_info
            if si is not None and si.on_wait and len(si.on_wait) > 1:
                waits = list(si.on_wait)
                for w in waits[:-1]:
                    n += 1
                    nop = mybir.InstNoOp(
                        name=f"waitsplit-{n}-{ins.name}",
                        sync_info=mybir.SyncInfo(on_wait=[w], on_update=[]),
                        bass_nofuse=True,
                        engine=ins.engine,
                    )
                    new_insts.append(nop)
                si.on_wait = waits[-1:]
            new_insts.append(ins)
        bb.instructions[:] = new_insts
    return n


# ---------------------------------------------------------------------------

N_NODES = 100000
F = 128
N_CORES = 8
ROWS_PER_CORE = N_NODES // N_CORES  # 12500
WIN = 128  # dest rows per window
P = 128  # edges per chunk
NBANK = 4
BANK = N_NODES // NBANK  # 25000 source rows per bank (< int16 max)
N_WIN = (ROWS_PER_CORE + WIN - 1) // WIN  # 98
GK = 8  # chunks per dma_gather instruction; 1024 descriptors is the ring max
SCRATCH = max(16384, 16 * GK * 128)  # SWDGE ring (16B/descriptor)
PAD_DST = 384.0  # miss value outside both iota ranges (exact in bf16)
ACT_STT_EVERY = 3  # every 3rd pure chunk's one-hot is built on the Act engine
G_BUFS = 6
SP_BUFS = 16
SW_BUFS = 6
PT_BUFS = 8
AGG_BUFS = 4
PA_BUFS = 4
PP_BUFS = 2
OUT_SLAB = 24  # windows per output writeback DMA

BF16 = ml_dtypes.bfloat16


def host_prep(x, W, edge_index):
    """Shared chunk grid + per-core edge placement.

    Returns per-core input arrays and the shared static layout:
    chunks per bank, per-chunk window/width info, per-window matmul lists.
    """
    row = np.asarray(edge_index[0], dtype=np.int64)
    col = np.asarray(edge_index[1], dtype=np.int64)
    x = np.asarray(x, dtype=np.float32)
    W = np.asarray(W, dtype=np.float32)

    deg = np.bincount(col, minlength=N_NODES)
    with np.errstate(divide="ignore"):
        dis = np.where(deg > 0, 1.0 / np.sqrt(deg.astype(np.float64)), 0.0).astype(
            np.float32
        )
    xs = (x * dis[:, None]).astype(BF16)  # dis[col] folded into sources

    core_of = row // ROWS_PER_CORE
    cnt = np.zeros((N_CORES, NBANK, N_WIN), dtype=np.int64)
    per_core = []
    rowmap = np.full((N_CORES, N_WIN * WIN), -1, dtype=np.int64)
    for k in range(N_CORES):
        m = core_of == k
        r = (row[m] - k * ROWS_PER_CORE).astype(np.int64)
        c = col[m].astype(np.int64)
        b = c // BANK
        # balance rows across windows so per-(bank, window) edge counts are
        # nearly equal on every core (shrinks the cross-core max padding):
        # greedy argmin of correlation with current window-bank sums.
        ebc = np.zeros((ROWS_PER_CORE, NBANK), dtype=np.int64)
        np.add.at(ebc, (r, b), 1)
        tot = ebc.sum(axis=1)
        order_rows = np.argsort(-tot, kind="stable")
        sums = np.zeros((N_WIN, NBANK), dtype=np.float64)
        space = np.full(N_WIN, WIN, dtype=np.int64)
        win_of = np.zeros(ROWS_PER_CORE, dtype=np.int64)
        slot_of = np.zeros(ROWS_PER_CORE, dtype=np.int64)
        for rr in order_rows:
            e = ebc[rr].astype(np.float64)
            score = sums @ e + 0.5 * (e @ e)
            score[space == 0] = np.inf
            wbest = int(np.argmin(score))
            win_of[rr] = wbest
            slot_of[rr] = WIN - space[wbest]
            sums[wbest] += e
            space[wbest] -= 1
            rowmap[k, wbest * WIN + slot_of[rr]] = k * ROWS_PER_CORE + rr
        w = win_of[r]
        dst_in_w = slot_of[r]
        order = np.lexsort((w, b))
        r, c, b, w, dst_in_w = (
            r[order], c[order], b[order], w[order], dst_in_w[order]
        )
        cnt[k] = np.stack(
            [np.bincount(w[b == bb], minlength=N_WIN) for bb in range(NBANK)]
        )
        per_core.append((r, c, b, w, dst_in_w))

    cap = cnt.max(axis=0)  # [NBANK, N_WIN] shared capacity per (bank, window)
    pos = np.zeros((NBANK, N_WIN + 1), dtype=np.int64)
    pos[:, 1:] = np.cumsum(cap, axis=1)
    bank_tot = pos[:, -1]
    chunks_b = ((bank_tot + P - 1) // P).astype(np.int64)
    slots_b = chunks_b * P

    # shared static chunk info: first window + wide flag, per (bank, chunk)
    first_win = []
    is_wide = []
    for bb in range(NBANK):
        fw = np.searchsorted(pos[bb], np.arange(chunks_b[bb]) * P, side="right") - 1
        fw = np.minimum(fw, N_WIN - 1)
        lw = (
            np.searchsorted(
                pos[bb], np.arange(chunks_b[bb]) * P + (P - 1), side="right"
            )
            - 1
        )
        lw = np.minimum(lw, N_WIN - 1)
        assert (lw - fw <= 1).all(), "chunk spans >2 windows"
        first_win.append(fw)
        is_wide.append(lw > fw)

    # per-window matmul segment lists: (bank, chunk, half)
    segments = [[] for _ in range(N_WIN)]
    for bb in range(NBANK):
        for cch in range(int(chunks_b[bb])):
            w0 = int(first_win[bb][cch])
            segments[w0].append((bb, cch, 0))
            if is_wide[bb][cch]:
                segments[w0 + 1].append((bb, cch, 1))

    # per-core edge placement into the shared grid
    c_total = int(chunks_b.sum())
    c_off = np.concatenate([[0], np.cumsum(chunks_b)[:-1]])  # bank -> col base
    idx16 = [np.zeros((N_CORES, int(s)), dtype=np.int16) for s in slots_b]
    dstcol = np.full((N_CORES, P, c_total), PAD_DST, dtype=np.float32)
    for k in range(N_CORES):
        r, c, b, w, dst_in_w = per_core[k]
        grp = b * N_WIN + w
        starts = np.zeros(NBANK * N_WIN, dtype=np.int64)
        gcnt = np.bincount(grp, minlength=NBANK * N_WIN)
        starts[1:] = np.cumsum(gcnt)[:-1]
        rank = np.arange(len(r)) - starts[grp]
        p = pos[b, w] + rank  # bank-local slot
        chunk = p >> 7
        part = p & (P - 1)
        fw_e = np.concatenate(first_win)[c_off[b] + chunk]
        dv = dst_in_w + 128 * (w - fw_e)
        assert ((w - fw_e) >= 0).all() and ((w - fw_e) <= 1).all()
        for bb in range(NBANK):
            m = b == bb
            idx16[bb][k, p[m]] = (c[m] - bb * BANK).astype(np.int16)
            dstcol[k, part[m], c_off[bb] + chunk[m]] = dv[m]

    # dma_gather index layout: idx i of a gather at 128-aligned offset o
    # lives at [partition (i-o)%16, slot (i-o)//16]; one global 16-wrap
    # layout serves every GK-chunk slice; replicate across 128 partitions.
    idx_tiles = []
    for bb in range(NBANK):
        s = int(slots_b[bb])
        t = idx16[bb].reshape(N_CORES, s // 16, 16).transpose(0, 2, 1)
        idx_tiles.append(np.ascontiguousarray(np.tile(t, (1, 8, 1))))

    dstcol_bf = dstcol.astype(BF16)
    dstcol_neg = np.ascontiguousarray(-dstcol)  # f32 bias for the Act path

    # dis of each core's dest rows, [partition(=slot-in-window), window]
    disrT = np.zeros((N_CORES, P, N_WIN), dtype=np.float32)
    for k in range(N_CORES):
        dpad = np.zeros(N_WIN * WIN, dtype=np.float32)
        mm = rowmap[k] >= 0
        dpad[mm] = dis[rowmap[k][mm]]
        disrT[k] = dpad.reshape(N_WIN, WIN).T

    wt = np.ascontiguousarray(W.T)  # rhs[f, of] = W[of, f] for the transposed proj
    iota = np.tile(np.arange(2 * WIN, dtype=np.float32), (P, 1)).astype(BF16)
    layout = dict(
        chunks_b=[int(v) for v in chunks_b],
        c_off=[int(v) for v in c_off],
        first_win=[fw.tolist() for fw in first_win],
        is_wide=[iw.tolist() for iw in is_wide],
        segments=segments,
        c_total=c_total,
    )
    return xs, wt, iota, idx_tiles, dstcol_bf, dstcol_neg, disrT, rowmap, layout


def build_program(layout):
    """Build the per-core Bass/Tile program (same for all cores)."""
    nc = bass.Bass(
        "TRN2",
        target_bir_lowering=False,
        debug=False,
        num_devices=1,
        dynamic_dma_scratch_size=SCRATCH,
    )
    dt = mybir.dt
    AF = mybir.ActivationFunctionType
    chunks_b = layout["chunks_b"]
    c_off = layout["c_off"]
    is_wide = layout["is_wide"]
    segments = layout["segments"]
    c_total = layout["c_total"]

    xs_d = [
        nc.dram_tensor(f"xs{b}", [BANK, F], dt.bfloat16, kind="ExternalInput")
        for b in range(NBANK)
    ]
    wt_d = nc.dram_tensor("wt", [F, F], dt.float32, kind="ExternalInput")
    iota_d = nc.dram_tensor("iota", [P, 2 * WIN], dt.bfloat16, kind="ExternalInput")
    idx_d = [
        nc.dram_tensor(
            f"idx{b}", [P, chunks_b[b] * P // 16], dt.int16, kind="ExternalInput"
        )
        for b in range(NBANK)
    ]
    sdst_d = nc.dram_tensor("sdst", [P, c_total], dt.bfloat16, kind="ExternalInput")
    disr_d = nc.dram_tensor("disr", [P, N_WIN], dt.float32, kind="ExternalInput")
    y_d = nc.dram_tensor("y", [P, N_WIN * F], dt.bfloat16, kind="ExternalOutput")

    n_batch = [(chunks_b[b] + GK - 1) // GK for b in range(NBANK)]

    with tile.TileContext(nc) as tc:
        with (
            tc.tile_pool(name="const", bufs=1) as const_pool,
            tc.tile_pool(name="out", bufs=1) as out_pool,
            tc.tile_pool(name="g0", bufs=G_BUFS) as g_pool0,
            tc.tile_pool(name="g1", bufs=G_BUFS) as g_pool1,
            tc.tile_pool(name="g2", bufs=G_BUFS) as g_pool2,
            tc.tile_pool(name="g3", bufs=G_BUFS) as g_pool3,
            tc.tile_pool(name="sp", bufs=SP_BUFS) as sp_pool,
            tc.tile_pool(name="sw", bufs=SW_BUFS) as sw_pool,
            tc.tile_pool(name="pt", bufs=PT_BUFS) as pt_pool,
            tc.tile_pool(name="aggsb", bufs=AGG_BUFS) as aggsb_pool,
            tc.tile_pool(name="psum_agg", bufs=PA_BUFS, space="PSUM") as pa_pool,
            tc.tile_pool(name="psum_proj", bufs=PP_BUFS, space="PSUM") as pp_pool,
        ):
            from concourse.library_config import mlp

            nc.gpsimd.load_library(mlp)
            g_pools = [g_pool0, g_pool1, g_pool2, g_pool3]
            wt_sb = const_pool.tile([F, F], dt.float32, tag="wt")
            nc.sync.dma_start(out=wt_sb[:], in_=wt_d.ap())
            iota_sb = const_pool.tile([P, 2 * WIN], dt.bfloat16, tag="iota")
            nc.sync.dma_start(out=iota_sb[:], in_=iota_d.ap())
            sdst_sb = const_pool.tile([P, c_total], dt.bfloat16, tag="sdst")
            nc.scalar.dma_start(out=sdst_sb[:], in_=sdst_d.ap())
            sdstn_sb = const_pool.tile([P, c_total], dt.float32, tag="sdstn")
            nc.vector.tensor_scalar(
                out=sdstn_sb[:],
                in0=sdst_sb[:],
                scalar1=-1.0,
                scalar2=None,
                op0=AluOpType.mult,
            )
            disr_sb = const_pool.tile([P, N_WIN], dt.float32, tag="disr")
            nc.scalar.dma_start(out=disr_sb[:], in_=disr_d.ap())
            idx_sb = []
            for b in range(NBANK):
                t = const_pool.tile(
                    [P, chunks_b[b] * P // 16], dt.int16, tag=f"idx{b}"
                )
                nc.sync.dma_start(out=t[:], in_=idx_d[b].ap())
                idx_sb.append(t)
            out_sb = out_pool.tile([P, N_WIN * F], dt.bfloat16, tag="out")

            g_tiles = [[None] * n_batch[b] for b in range(NBANK)]
            s_tiles = {}  # (bank, chunk) -> tile
            reg_cache = {}
            pure_ctr = 0

            def nreg(v):
                if v not in reg_cache:
                    reg_cache[v] = nc.gpsimd.to_reg(v)
                return reg_cache[v]

            def emit_gather(b, bi):
                c_lo = bi * GK
                c_n = min(GK, chunks_b[b] - c_lo)
                g_t = g_pools[b].tile([P, GK, F], dt.bfloat16, tag=f"g{b}")
                nc.gpsimd.dma_gather(
                    g_t[:, :c_n, :],
                    xs_d[b].ap(),
                    idx_sb[b][:, c_lo * 8 : (c_lo + c_n) * 8],
                    c_n * P,
                    nreg(c_n * P),
                    F,
                )
                g_tiles[b][bi] = g_t

            def emit_s(b, cch):
                nonlocal pure_ctr
                wide = is_wide[b][cch]
                width = 2 * WIN if wide else WIN
                col = c_off[b] + cch
                on_act = False
                if not wide and ACT_STT_EVERY:
                    pure_ctr += 1
                    on_act = pure_ctr % ACT_STT_EVERY == 0
                pool_ = sw_pool if wide else sp_pool
                s_t = pool_.tile([P, width], dt.bfloat16, tag="sw" if wide else "sp")
                if on_act:
                    # |iota - dst| then relu(1 - z): exact one-hot for ints
                    t_t = pt_pool.tile([P, WIN], dt.bfloat16, tag="pt")
                    nc.scalar.activation(
                        out=t_t[:],
                        in_=iota_sb[:, :WIN],
                        func=AF.Abs,
                        bias=sdstn_sb[:, col : col + 1],
                    )
                    nc.scalar.activation(
                        out=s_t[:], in_=t_t[:], func=AF.Relu, bias=1.0, scale=-1.0
                    )
                else:
                    nc.vector.scalar_tensor_tensor(
                        out=s_t[:],
                        in0=iota_sb[:, :width],
                        scalar=sdst_sb[:, col : col + 1],
                        in1=iota_sb[:, :width],
                        op0=AluOpType.is_equal,
                        op1=AluOpType.bypass,
                    )
                s_tiles[(b, cch)] = s_t

            for w in range(N_WIN):
                segs = segments[w]
                if not segs:
                    continue
                agg = pa_pool.tile([F, WIN], dt.float32, tag="agg")
                n_s = len(segs)
                for j, (b, cch, half) in enumerate(segs):
                    bi, cb = divmod(cch, GK)
                    if g_tiles[b][bi] is None:
                        emit_gather(b, bi)
                    if half == 0:
                        emit_s(b, cch)
                    s_t = s_tiles[(b, cch)]
                    nc.tensor.matmul(
                        agg[:],
                        lhsT=g_tiles[b][bi][:, cb, :],
                        rhs=s_t[:, half * WIN : (half + 1) * WIN],
                        start=(j == 0),
                        stop=(j == n_s - 1),
                    )
                agg_sb = aggsb_pool.tile([F, WIN], dt.float32, tag="aggsb")
                nc.scalar.activation(out=agg_sb[:], in_=agg[:], func=AF.Copy)
                proj = pp_pool.tile([WIN, F], dt.float32, tag="proj")
                nc.tensor.matmul(
                    proj[:], lhsT=agg_sb[:], rhs=wt_sb[:], start=True, stop=True
                )
                # PSUM [dest, feat] -> out, scaled by dis_row (per-partition)
                nc.scalar.activation(
                    out=out_sb[:, w * F : (w + 1) * F],
                    in_=proj[:],
                    func=AF.Copy,
                    scale=disr_sb[:, w : w + 1],
                )
                # slab writeback: overlap output DMA with later windows
                if (w + 1) % OUT_SLAB == 0 or w == N_WIN - 1:
                    w_lo = (w // OUT_SLAB) * OUT_SLAB
                    nc.sync.dma_start(
                        out=y_d.ap()[:, w_lo * F : (w + 1) * F],
                        in_=out_sb[:, w_lo * F : (w + 1) * F],
                    )
    split_multi_waits(nc)
    mybir.codegen_inst_isa_subclasses(nc)
    return nc


def make_in_maps(xs, wt, iota, idx_tiles, dstcol, dstneg, disrT):
    in_maps = []
    for k in range(N_CORES):
        m = {
            "wt": wt,
            "iota": iota,
            "sdst": dstcol[k],
            "disr": disrT[k],
        }
        for b in range(NBANK):
            m[f"xs{b}"] = xs[b * BANK : (b + 1) * BANK]
            m[f"idx{b}"] = idx_tiles[b][k]
        in_maps.append(m)
    return in_maps


def unshard(res, rowmap):
    out = np.zeros((N_NODES, F), dtype=np.float32)
    for k in range(N_CORES):
        y_t = res.results[k]["y"].astype(np.float32)  # [P, N_WIN*F]
        o = y_t.reshape(P, N_WIN, F).transpose(1, 0, 2).reshape(N_WIN * WIN, F)
        mm = rowmap[k] >= 0
        out[rowmap[k][mm]] = o[mm]
    return out


def kernel(x, W, edge_index):
    xs, wt, iota, idx_tiles, dstcol, dstneg, disrT, rowmap, layout = host_prep(
        x, W, edge_index
    )
    nc = build_program(layout)
    in_maps = make_in_maps(xs, wt, iota, idx_tiles, dstcol, dstneg, disrT)
    res = run_bass_kernel_spmd(nc, in_maps, core_ids=list(range(N_CORES)))
    return unshard(res, rowmap)
